# revision 1
# baseline (speedup 1.0000x reference)
"""Trainium2 Bass kernel for nn_EnerG (3-layer NNConv GNN + sum-pool + MLP).

Strategy (8 cores, SPMD):
  - Edges sharded across cores (4000/core, padded to 4096), sorted by dst.
  - Node features replicated; per-layer aggregation via indirect-DMA-add
    scatter into DRAM, AllReduce across cores, replicated node update.
  - Per-edge weight tensors (the big [E,64,128] intermediate) are fused in
    SBUF: PE generates pre-activations, ACT applies leaky-relu on PSUM
    evacuation, DVE contracts against gathered source features.
"""
import sys

sys.path.insert(0, "/opt/trn_rl_repo")

import numpy as np

import concourse.bass as bass
import concourse.tile as tile
from concourse import bacc, mybir
from concourse.bass_utils import run_bass_kernel_spmd
from concourse.masks import make_identity

F32 = mybir.dt.float32
BF16 = mybir.dt.bfloat16
I32 = mybir.dt.int32
AF = mybir.ActivationFunctionType
OP = mybir.AluOpType

N_CORES = 8
N = 8000
E = 32000
G = 32
NP = 8064            # padded nodes (63 * 128)
EC = 4096            # padded edges per core (32 * 128)
NT = NP // 128       # 63 node tiles
ET = EC // 128       # 32 edge tiles
JUNK = NP - 1        # junk node for dead scatter slots

# layer dims: (in_ch, out_ch)
LAYERS = [(4, 8), (8, 64), (64, 128)]


def build_graph(reps: int = 1, debug: bool = False, ablate=()):
    nc = bacc.Bacc("TRN2", target_bir_lowering=False, debug=False,
                   num_devices=N_CORES)

    def din(name, shape, dt=F32):
        return nc.dram_tensor(name, shape, dt, kind="ExternalInput").ap()

    # per-core edge data
    eaT = din("eaT", [4, EC])                 # edge attrs ^T + ones row
    xsrc = din("xsrc", [EC, 4])               # x[src] (layer-1 H)
    srcidx = din("srcidx", [EC, 1], I32)
    dstw = din("dstw", [EC, 1], I32)          # dedup'd dst (else JUNK)
    mcomb = din("mcomb", [128, EC])           # per-tile combine matrices
    # replicated node/graph data
    xaT = din("xaT", [5, NP])                 # x^T + ones row
    batchg = din("batchg", [NP, 1], I32)
    # weights
    w1a = [din(f"w1a{l}", [4, 64]) for l in range(3)]
    w2a = [din(f"w2a{l}", [65, LAYERS[l][0] * LAYERS[l][1] + 0]) for l in range(3)]
    r_a = [din("r1a", [5, 8]), din("r2a", [9, 64]), din("r3a", [65, 128])]
    fc1 = din("fc1", [128, 128])
    fc1b = din("fc1b", [128, 1])
    fc2 = din("fc2", [128, 64])
    fc2b = din("fc2b", [64, 1])
    fc3 = din("fc3", [64, 1])
    fc3b = din("fc3b", [32, 1])

    out = nc.dram_tensor("out", [G, 1], F32, kind="ExternalOutput").ap()
    dbg = {}
    if debug:
        dbg["efT1"] = nc.dram_tensor("efT1_o", [65, EC], F32, kind="ExternalOutput").ap()
        for l in range(3):
            dbg[f"msgs{l}"] = nc.dram_tensor(f"msgs{l}_o", [EC, LAYERS[l][1]], F32,
                                             kind="ExternalOutput").ap()
            dbg[f"aggl{l}"] = nc.dram_tensor(f"aggl{l}_o", [NP, LAYERS[l][1]], F32,
                                             kind="ExternalOutput").ap()
        dbg["h1"] = nc.dram_tensor("h1_o", [NP, 8], F32, kind="ExternalOutput").ap()
        dbg["h2"] = nc.dram_tensor("h2_o", [NP, 64], F32, kind="ExternalOutput").ap()
        dbg["pool"] = nc.dram_tensor("pool_o", [128, G], F32, kind="ExternalOutput").ap()

    # internal DRAM
    h_dram = [None,
              nc.dram_tensor("h1d", [NP, 8], F32).ap(),
              nc.dram_tensor("h2d", [NP, 64], F32).ap()]
    agg_l = [nc.dram_tensor(f"agg{l}", [NP, LAYERS[l][1]], F32).ap()
             for l in range(3)]
    hpT_dram = [None,
                nc.dram_tensor("h1pTd", [9, NP], F32).ap(),
                nc.dram_tensor("h2pTd", [65, NP], F32).ap()]
    agg_g = [nc.dram_tensor(f"aggg{l}", [NP, LAYERS[l][1]], F32,
                            addr_space="Shared").ap() for l in range(3)]
    pool_l = nc.dram_tensor("pooll", [128, G], F32).ap()
    pool_g = nc.dram_tensor("poolg", [128, G], F32, addr_space="Shared").ap()

    groups = [list(range(N_CORES))]

    with tile.TileContext(nc) as tc:
        _build_body(nc, tc, reps, locals(), dbg, ablate)
    nc.compile()
    return nc


def _build_body(nc, tc, reps, v, dbg=None, ablate=()):
    dbg = dbg or {}
    eaT, xsrc, srcidx, dstw, mcomb = v["eaT"], v["xsrc"], v["srcidx"], v["dstw"], v["mcomb"]
    xaT, batchg = v["xaT"], v["batchg"]
    w1a, w2a, r_a = v["w1a"], v["w2a"], v["r_a"]
    fc1, fc1b, fc2, fc2b, fc3, fc3b = v["fc1"], v["fc1b"], v["fc2"], v["fc2b"], v["fc3"], v["fc3b"]
    out, h_dram, agg_l, agg_g = v["out"], v["h_dram"], v["agg_l"], v["agg_g"]
    hpT_dram = v["hpT_dram"]
    pool_l, pool_g, groups = v["pool_l"], v["pool_g"], v["groups"]

    import contextlib
    ctx = contextlib.ExitStack()
    with ctx:
        persist = ctx.enter_context(tc.tile_pool(name="persist", bufs=1))
        wpool = ctx.enter_context(tc.tile_pool(name="wpool", bufs=2))
        small = ctx.enter_context(tc.tile_pool(name="small", bufs=4))
        psum_w = ctx.enter_context(tc.tile_pool(name="psum_w", bufs=2, space="PSUM"))
        psum_m = ctx.enter_context(tc.tile_pool(name="psum_m", bufs=4, space="PSUM"))

        # ---- persistent SBUF loads ----
        ident = persist.tile([128, 128], F32, name="ident")
        make_identity(nc, ident[:])
        ident_bf = persist.tile([128, 128], BF16, name="ident_bf")
        nc.vector.tensor_copy(ident_bf[:], ident[:])
        mcomb_sb = persist.tile([128, EC], F32, name="mcomb_sb")
        nc.sync.dma_start(mcomb_sb[:], mcomb[:])
        w1a_sb = []
        w2a_sb = []
        for l in range(3):
            t1 = persist.tile([4, 64], F32, name=f"w1a_sb{l}")
            nc.sync.dma_start(t1[:], w1a[l][:])
            w1a_sb.append(t1)
            c = LAYERS[l][0] * LAYERS[l][1]
            t2 = persist.tile([65, c], F32, name=f"w2a_sb{l}")
            nc.sync.dma_start(t2[:], w2a[l][:])
            w2a_sb.append(t2)
        ra_sb = []
        for l, shp in enumerate([[5, 8], [9, 64], [65, 128]]):
            t = persist.tile(shp, F32, name=f"ra_sb{l}")
            nc.sync.dma_start(t[:], r_a[l][:])
            ra_sb.append(t)
        fc1_sb = persist.tile([128, 128], F32, name="fc1_sb")
        nc.sync.dma_start(fc1_sb[:], fc1[:])
        fc1b_sb = persist.tile([128, 1], F32, name="fc1b_sb")
        nc.sync.dma_start(fc1b_sb[:], fc1b[:])
        fc2_sb = persist.tile([128, 64], F32, name="fc2_sb")
        nc.sync.dma_start(fc2_sb[:], fc2[:])
        fc2b_sb = persist.tile([64, 1], F32, name="fc2b_sb")
        nc.sync.dma_start(fc2b_sb[:], fc2b[:])
        fc3_sb = persist.tile([64, 1], F32, name="fc3_sb")
        nc.sync.dma_start(fc3_sb[:], fc3[:])
        fc3b_sb = persist.tile([32, 1], F32, name="fc3b_sb")
        nc.sync.dma_start(fc3b_sb[:], fc3b[:])
        zero_sb = persist.tile([128, 128], F32, name="zero_sb")
        nc.vector.memset(zero_sb[:], 0.0)
        iota_f = persist.tile([128, 128], F32, name="iota_f")
        iota_i = persist.tile([128, 128], I32, name="iota_i")
        nc.gpsimd.iota(iota_i[:], pattern=[[1, 128]], channel_multiplier=0)
        nc.vector.tensor_copy(iota_f[:], iota_i[:])

        # gathered H per layer lives here ([128, 32*in_ch])
        h_gath = persist.tile([128, ET * 64], F32, name="h_gath")
        # index tiles: column t holds tile t's rows
        srcidx_sb = persist.tile([128, ET], I32, name="srcidx_sb")
        nc.sync.dma_start(srcidx_sb[:], srcidx[:].rearrange("(t p) one -> p (t one)", p=128))
        dstw_sb = persist.tile([128, ET], I32, name="dstw_sb")
        nc.sync.dma_start(dstw_sb[:], dstw[:].rearrange("(t p) one -> p (t one)", p=128))
        efT = persist.tile([65, EC], F32, name="efT")

        for rep in range(reps):
            poolT_ps = None
            for l in range(3):
                for j in range(NT):
                    nc.sync.dma_start(agg_l[l][128 * j:128 * (j + 1), :],
                                      zero_sb[:, :LAYERS[l][1]])
            for l in range(3):
                cin, cout = LAYERS[l]
                ncols = cin * cout

                # ---- edge-net hidden: efT = Prelu(w1a^T @ eaT), + ones row
                for ch in range(EC // 512):
                    ea_t = small.tile([4, 512], F32, name="ea_t")
                    nc.sync.dma_start(ea_t[:], eaT[:, 512 * ch:512 * (ch + 1)])
                    ps = psum_m.tile([128, 512], F32, name="efps", tag="psm")
                    nc.tensor.matmul(ps[:64, :], lhsT=w1a_sb[l][:],
                                     rhs=ea_t[:], start=True, stop=True)
                    nc.scalar.activation(efT[0:64, 512 * ch:512 * (ch + 1)],
                                         ps[:64, :], AF.Prelu, alpha=0.1)
                nc.vector.memset(efT[64:65, :], 1.0)
                if l == 0 and "efT1" in dbg:
                    for ch in range(EC // 512):
                        sn = small.tile([65, 512], F32, name="efsn")
                        nc.vector.tensor_copy(sn[:], efT[:, 512 * ch:512 * (ch + 1)])
                        nc.sync.dma_start(dbg["efT1"][:, 512 * ch:512 * (ch + 1)], sn[:])

                # ---- gather H (source features) ----
                if l == 0:
                    for t in range(ET):
                        nc.sync.dma_start(h_gath[:, 64 * t:64 * t + cin],
                                          xsrc[128 * t:128 * (t + 1), :])
                else:
                    for t in range(ET):
                        if "nogath" in ablate:
                            nc.sync.dma_start(h_gath[:, 64 * t:64 * t + cin],
                                              h_dram[l][:128, :][0:128, :])
                            continue
                        nc.gpsimd.indirect_dma_start(
                            out=h_gath[:, 64 * t:64 * t + cin],
                            out_offset=None,
                            in_=h_dram[l][:],
                            in_offset=bass.IndirectOffsetOnAxis(
                                ap=srcidx_sb[:, t:t + 1], axis=0),
                        )

                # ---- edge stream ----
                for t in range(ET):
                    acc = small.tile([128, 128], F32, name="acc")
                    hsl = h_gath[:, 64 * t:64 * t + cin]
                    if l == 2:
                        for half in range(2):
                            base = 4096 * half
                            w_sb = wpool.tile([128, 4096], F32, name="w_sb")
                            for gb in range(0, 4096, 1024):
                                ps = psum_w.tile([128, 1024], F32, name="wps")
                                for sb in range(0, 1024, 512):
                                    nc.tensor.matmul(
                                        ps[:, sb:sb + 512],
                                        lhsT=efT[:, 128 * t:128 * (t + 1)],
                                        rhs=w2a_sb[l][:, base + gb + sb:
                                                       base + gb + sb + 512],
                                        start=True, stop=True)
                                nc.scalar.activation(w_sb[:, gb:gb + 1024],
                                                     ps[:], AF.Prelu,
                                                     alpha=0.1)
                            for i in range(32 * half, 32 * half + 32):
                                src_ap = w_sb[:, i * cout - base:
                                              (i + 1) * cout - base]
                                if i == 0:
                                    nc.vector.tensor_scalar(
                                        acc[:, :cout], src_ap, hsl[:, 0:1],
                                        None, op0=OP.mult)
                                else:
                                    nc.vector.scalar_tensor_tensor(
                                        out=acc[:, :cout], in0=src_ap,
                                        scalar=hsl[:, i:i + 1],
                                        in1=acc[:, :cout],
                                        op0=OP.mult, op1=OP.add)
                    else:
                        w_sb = wpool.tile([128, 4096], F32, name="w_sbf")
                        # W-gen (PE) + leaky evacuation (ACT)
                        for gb in range(0, ncols, 1024):
                            gcols = min(1024, ncols - gb)
                            ps = psum_w.tile([128, 1024], F32, name="wps")
                            for sb in range(0, gcols, 512):
                                scols = min(512, gcols - sb)
                                nc.tensor.matmul(
                                    ps[:, sb:sb + scols],
                                    lhsT=efT[:, 128 * t:128 * (t + 1)],
                                    rhs=w2a_sb[l][:, gb + sb:gb + sb + scols],
                                    start=True, stop=True)
                            nc.scalar.activation(w_sb[:, gb:gb + gcols],
                                                 ps[:, :gcols], AF.Prelu,
                                                 alpha=0.1)
                        # MAC (DVE): acc[e, o] += W[e, i*cout+o] * H[e, i]
                        for i in range(cin):
                            src_ap = w_sb[:, i * cout:(i + 1) * cout]
                            if i == 0:
                                nc.vector.tensor_scalar(
                                    acc[:, :cout], src_ap, hsl[:, 0:1], None,
                                    op0=OP.mult)
                            else:
                                nc.vector.scalar_tensor_tensor(
                                    out=acc[:, :cout], in0=src_ap,
                                    scalar=hsl[:, i:i + 1], in1=acc[:, :cout],
                                    op0=OP.mult, op1=OP.add)

                    if f"msgs{l}" in dbg:
                        nc.sync.dma_start(dbg[f"msgs{l}"][128 * t:128 * (t + 1), :],
                                          acc[:, :cout])
                    # combine duplicates within tile + scatter-add to DRAM
                    cps = psum_m.tile([128, 512], F32, name="cps", tag="psm")
                    nc.tensor.matmul(cps[:, :cout],
                                     lhsT=mcomb_sb[:, 128 * t:128 * (t + 1)],
                                     rhs=acc[:, :cout], start=True, stop=True)
                    msg2 = small.tile([128, 128], F32, name="msg2")
                    nc.vector.tensor_copy(msg2[:, :cout], cps[:, :cout])
                    if "noscat" not in ablate:
                        nc.gpsimd.indirect_dma_start(
                            out=agg_l[l][:],
                            out_offset=bass.IndirectOffsetOnAxis(
                                ap=dstw_sb[:, t:t + 1], axis=0),
                            in_=msg2[:, :cout], in_offset=None,
                            compute_op=OP.add)
                    else:
                        nc.sync.dma_start(
                            agg_l[l][128 * t:128 * (t + 1), :],
                            msg2[:, :cout])

                if f"aggl{l}" in dbg:
                    for j in range(NT):
                        sn = small.tile([128, 128], F32, name="aggsn")
                        nc.sync.dma_start(sn[:, :cout],
                                          agg_l[l][128 * j:128 * (j + 1), :])
                        nc.sync.dma_start(dbg[f"aggl{l}"][128 * j:128 * (j + 1), :],
                                          sn[:, :cout])
                # ---- AllReduce ----
                if "nocoll" not in ablate:
                    nc.gpsimd.collective_compute(
                        "AllReduce", OP.add, replica_groups=groups,
                        ins=[agg_l[l][:]], outs=[agg_g[l][:]])

                # ---- node update: h = Prelu(agg + h_prev' @ root') ----
                prevT_dram = xaT if l == 0 else hpT_dram[l]
                kdim = [5, 9, 65][l]
                for j in range(NT):
                    pvT = small.tile([65, 128], F32, name="pvT")
                    nc.sync.dma_start(pvT[:kdim, :],
                                      prevT_dram[:, 128 * j:128 * (j + 1)])
                    ps = psum_m.tile([128, 512], F32, name="hups", tag="psm")
                    nc.tensor.matmul(ps[:, :cout],
                                     lhsT=pvT[:kdim, :],
                                     rhs=ra_sb[l][:], start=True, stop=False)
                    ag = small.tile([128, 128], F32, name="ag")
                    agsrc = agg_l[l] if "nocoll" in ablate else agg_g[l]
                    nc.sync.dma_start(ag[:, :cout],
                                      agsrc[128 * j:128 * (j + 1), :])
                    nc.tensor.matmul(ps[:, :cout], lhsT=ident[:],
                                     rhs=ag[:, :cout], start=False, stop=True)
                    if l < 2:
                        # h tile (+ones col), write DRAM + build transposed form
                        ht = small.tile([128, 72], F32, name="ht")
                        nc.scalar.activation(ht[:, :cout], ps[:, :cout],
                                             AF.Prelu, alpha=0.1)
                        nc.sync.dma_start(h_dram[l + 1][128 * j:128 * (j + 1), :],
                                          ht[:, :cout])
                        if f"h{l + 1}" in dbg:
                            nc.sync.dma_start(
                                dbg[f"h{l + 1}"][128 * j:128 * (j + 1), :],
                                ht[:, :cout])
                        nc.vector.memset(ht[:, cout:cout + 1], 1.0)
                        tps = psum_m.tile([128, 512], F32, name="tps", tag="psm")
                        nc.tensor.transpose(tps[:cout + 1, :128],
                                            ht[:, :cout + 1], ident[:])
                        hT_sb = small.tile([65, 128], F32, name="hT_sb")
                        nc.vector.tensor_copy(hT_sb[:cout + 1, :],
                                              tps[:cout + 1, :128])
                        nc.sync.dma_start(
                            hpT_dram[l + 1][:, 128 * j:128 * (j + 1)],
                            hT_sb[:cout + 1, :])
                    else:
                        # layer 3: h3 tile feeds pooling directly
                        ht = small.tile([128, 128], F32, name="ht3")
                        nc.scalar.activation(ht[:, :cout], ps[:, :cout],
                                             AF.Prelu, alpha=0.1)
                        bg = small.tile([128, 1], I32, name="bg")
                        nc.sync.dma_start(bg[:], batchg[128 * j:128 * (j + 1), :])
                        bgf = small.tile([128, 1], F32, name="bgf")
                        nc.vector.tensor_copy(bgf[:], bg[:])
                        oh = small.tile([128, G], F32, name="oh")
                        nc.vector.tensor_scalar(oh[:], iota_f[:, :G], bgf[:, :1],
                                                None, op0=OP.is_equal)
                        if j == 0:
                            poolT_ps = psum_m.tile([128, 512], F32, name="poolps", tag="psm")
                        nc.tensor.matmul(poolT_ps[:, :G], lhsT=ht[:, :cout],
                                         rhs=oh[:], start=(j == 0),
                                         stop=(j == NT - 1))

            # ---- MLP (all transposed; pool already complete per-core) ----
            pg = small.tile([128, G], F32, name="pg")
            nc.vector.tensor_copy(pg[:], poolT_ps[:, :G])
            if "pool" in dbg:
                nc.sync.dma_start(dbg["pool"][:], pg[:])

            m1 = psum_m.tile([128, 512], F32, name="m1", tag="psm")
            nc.tensor.matmul(m1[:, :G], lhsT=fc1_sb[:], rhs=pg[:], start=True, stop=True)
            t1 = small.tile([128, G], F32, name="t1")
            nc.scalar.activation(t1[:], m1[:, :G], AF.Prelu, bias=fc1b_sb[:, :1],
                                 alpha=0.1)
            m2 = psum_m.tile([128, 512], F32, name="m2", tag="psm")
            nc.tensor.matmul(m2[:64, :G], lhsT=fc2_sb[:], rhs=t1[:], start=True, stop=True)
            t2 = small.tile([64, G], F32, name="t2")
            nc.scalar.activation(t2[:], m2[:64, :G], AF.Prelu, bias=fc2b_sb[:, :1],
                                 alpha=0.1)
            # final: out[g] = t2[:, g] . fc3 + b  (transposed matmul -> [32, 1])
            m3 = psum_m.tile([128, 512], F32, name="m3", tag="psm")
            nc.tensor.matmul(m3[:G, :1], lhsT=t2[:], rhs=fc3_sb[:], start=True, stop=True)
            ot = small.tile([G, 1], F32, name="ot")
            nc.vector.scalar_tensor_tensor(out=ot[:], in0=m3[:G, :1], scalar=1.0,
                                           in1=fc3b_sb[:G, :], op0=OP.mult,
                                           op1=OP.add)
            nc.sync.dma_start(out[:], ot[:])


# ---------------------------------------------------------------------------
# host side
# ---------------------------------------------------------------------------

_CACHE = {}


def _prep_core_inputs(x, edge_index, batch_index, p):
    """Build per-core + replicated input arrays."""
    ec = E // N_CORES
    xs = x.astype(np.float32)
    xaT = np.zeros((5, NP), np.float32)
    xaT[:4, :N] = xs.T
    xaT[4, :] = 1.0
    bg = np.full((NP, 1), 64, np.int32)
    bg[:N, 0] = batch_index.astype(np.int32)

    def aug_w(w, b):
        return np.vstack([w, b[None, :]]).astype(np.float32)

    rep = dict(
        xaT=xaT, batchg=bg,
        w1a0=aug_w(p["en1_w1"], p["en1_b1"]), w2a0=aug_w(p["en1_w2"], p["en1_b2"]),
        w1a1=aug_w(p["en2_w1"], p["en2_b1"]), w2a1=aug_w(p["en2_w2"], p["en2_b2"]),
        w1a2=aug_w(p["en3_w1"], p["en3_b1"]), w2a2=aug_w(p["en3_w2"], p["en3_b2"]),
        r1a=aug_w(p["root1"], p["cb1"]), r2a=aug_w(p["root2"], p["cb2"]),
        r3a=aug_w(p["root3"], p["cb3"]),
        fc1=p["fc1_w"].astype(np.float32),
        fc1b=p["fc1_b"].reshape(128, 1).astype(np.float32),
        fc2=p["fc2_w"].astype(np.float32),
        fc2b=p["fc2_b"].reshape(64, 1).astype(np.float32),
        fc3=p["fc3_w"].astype(np.float32),
        fc3b=np.repeat(p["fc3_b"].reshape(1, 1), G, 0).astype(np.float32),
    )

    in_maps = []
    for c in range(N_CORES):
        sl = slice(c * ec, (c + 1) * ec)
        src = edge_index[0, sl].astype(np.int64)
        dst = edge_index[1, sl].astype(np.int64)
        order = np.argsort(dst, kind="stable")
        src, dst = src[order], dst[order]
        nreal = len(src)

        ea = (xs[dst] - xs[src])[:, 1:]                     # [ec, 3]
        eaT = np.zeros((4, EC), np.float32)
        eaT[:3, :nreal] = ea.T
        eaT[3, :] = 1.0
        xsrc = np.zeros((EC, 4), np.float32)
        xsrc[:nreal] = xs[src]
        srcidx = np.zeros((EC, 1), np.int32)
        srcidx[:nreal, 0] = src

        dstw = np.full((EC, 1), JUNK, np.int32)
        mcomb = np.zeros((128, EC), np.float32)
        for t in range(ET):
            lo = t * 128
            d_tile = dst[lo:min(lo + 128, nreal)] if lo < nreal else np.array([])
            first = {}
            for i, d in enumerate(d_tile):
                if d in first:
                    mcomb[i, lo + first[d]] = 1.0
                else:
                    first[d] = i
                    mcomb[i, lo + i] = 1.0
                    dstw[lo + i, 0] = d
        in_maps.append(dict(eaT=eaT, xsrc=xsrc, srcidx=srcidx, dstw=dstw,
                            mcomb=mcomb, **rep))
    return in_maps


def kernel(x, edge_index, batch_index, **p):
    if "nc" not in _CACHE:
        _CACHE["nc"] = build_graph(reps=1)
    nc = _CACHE["nc"]
    in_maps = _prep_core_inputs(np.asarray(x), np.asarray(edge_index),
                                np.asarray(batch_index),
                                {k: np.asarray(v) for k, v in p.items()})
    res = run_bass_kernel_spmd(nc, in_maps, list(range(N_CORES)))
    return res.results[0]["out"].astype(np.float32)

